# revision 1
# baseline (speedup 1.0000x reference)
"""MAE ViT encoder (nn_MaskedAutoencoderViT) Trainium2 Bass kernel.

Strategy: data-parallel over batch (16 images -> 8 cores x 2 images).
Feature-major activation layout on chip: activations stored transposed as
[128 partitions (d chunk), 6 chunks, 152 tokens] so every matmul is
weight-stationary (lhsT = 128x128 weight tile, rhs = activation columns)
with zero on-device transposes.  Attention is computed in transposed form
(S^T = (K^T)-stationary @ Q^T), softmax uses the structure
exp(att)/ (sum + 1e-9) (the reference's global-max subtraction cancels in
the normalization up to ~1e-10 relative, far below fp32 noise).
Matmul operands in fp16 (full PE rate, 11-bit mantissa), accumulation and
residual stream in fp32.

Host side does only data marshalling: noise argsort, patch gather,
pos-embed gathers, weight transposition + fp16 cast.
"""
import numpy as np
from contextlib import ExitStack

import concourse.bass as bass
import concourse.bacc as bacc
import concourse.mybir as mybir
import concourse.tile as tile
import bass_rust as _bass_rust
from concourse.bass_utils import run_bass_kernel_spmd
from concourse.hw_specs import get_activation_tables


class _Bacc(bacc.Bacc):
    """Bacc whose ACT-table-load pass prefers multi-function sets.

    The stock pass picks the first table set containing each activation
    function, which sends Ln to `natural_log` and Exp to `exp_and_others`
    and thrashes the table RAM inside every layernorm.  Reordering the
    set dict so `natural_log_exp_and_others` comes first makes Ln and Exp
    share one resident set (2 loads per layer total: exp-set <-> gelu-set).
    """

    def insert_act_table_loads(self):
        has_activation = any(
            isinstance(i, mybir.InstActivation)
            for b in self.main_func.blocks
            for i in b.instructions
        )
        if not has_activation:
            return
        tabs = dict(get_activation_tables(self.m.arch))
        pref = ["natural_log_exp_and_others", "gelu_and_others"]
        ordered = {k: tabs[k] for k in pref if k in tabs}
        ordered.update({k: v for k, v in tabs.items() if k not in ordered})
        _bass_rust.insert_act_table_loads(self, list(ordered.items()))

F16 = mybir.dt.float16
F32 = mybir.dt.float32
AF = mybir.ActivationFunctionType
OP = mybir.AluOpType

# --- model config (hardcoded from the problem spec) ---
B, C_IN, H_IN, W_IN = 16, 1, 12, 2500
P_, Q_ = 1, 100
D, NH, DEPTH = 768, 12, 12
GH, GW = 12, 25
L = GH * GW                      # 300
LEN_KEEP = 75
HD = D // NH                     # 64
SCALE = HD ** -0.5               # 0.125
EPS_LN = 1e-5
MLP = 4 * D                      # 3072

NCORES = 8
BL = B // NCORES                 # 2 images per core
KT = 1 + LEN_KEEP                # 76 tokens per image
T = BL * KT                      # 152 token columns per core
NCH = D // 128                   # 6 feature chunks
MCH = MLP // 128                 # 24 mlp chunks
PIX = P_ * Q_                    # 100 pixels per patch


def bfree(ap, n, at=1):
    """Insert a 0-step (broadcast) free dim of size n at position `at`."""
    new_ap = list(ap.ap[:at]) + [[0, n]] + list(ap.ap[at:])
    return bass.AP(tensor=ap.tensor, offset=ap.offset, ap=new_ap)


def build(depth=DEPTH):
    nc = bacc.Bacc("TRN2", target_bir_lowering=False, debug=False,
                   num_devices=NCORES)

    # DRAM I/O
    patchesT = nc.dram_tensor("patchesT", [PIX, T], F16, kind="ExternalInput").ap()
    posT = nc.dram_tensor("posT", [NCH, 128, T], F32, kind="ExternalInput").ap()
    mvec = nc.dram_tensor("mvec", [BL, KT], F16, kind="ExternalInput").ap()
    mvecf = nc.dram_tensor("mvecf", [1, BL, 2, 6 * KT], F32, kind="ExternalInput").ap()
    wpatchT = nc.dram_tensor("wpatchT", [PIX, D], F16, kind="ExternalInput").ap()
    wqkvT = nc.dram_tensor("wqkvT", [depth, D, 3 * D], F16, kind="ExternalInput").ap()
    wprojT = nc.dram_tensor("wprojT", [depth, D, D], F16, kind="ExternalInput").ap()
    wfc1T = nc.dram_tensor("wfc1T", [depth, D, MLP], F16, kind="ExternalInput").ap()
    wfc2T = nc.dram_tensor("wfc2T", [depth, MLP, D], F16, kind="ExternalInput").ap()
    wsqn = nc.dram_tensor("wsqn", [depth, 2 * D], F16, kind="ExternalInput").ap()
    wsf1n = nc.dram_tensor("wsf1n", [depth, MLP], F16, kind="ExternalInput").ap()
    out_d = nc.dram_tensor("out", [NCH, 128, T], F32, kind="ExternalOutput").ap()

    with tile.TileContext(nc) as tc, ExitStack() as ctx:
        pool = lambda name, bufs, **kw: ctx.enter_context(
            tc.tile_pool(name=name, bufs=bufs, **kw))

        const = pool("const", 1)
        hp = pool("hp", 1)
        lnp = pool("lnp", 1)
        yp = pool("yp", 2)
        tmpp = pool("tmpp", 1)
        qkp = pool("qkp", 1)
        vp = pool("vp", 2)
        ep = pool("ep", 2)
        otp = pool("otp", 1)
        gp = pool("gp", 1)
        bcp = pool("bcp", 2)
        tinyp = pool("tinyp", 5)
        medp = pool("medp", 2)
        wsump = pool("wsump", 2)
        wqkvp = pool("wqkvp", 7)
        wprojp = pool("wprojp", 7)
        wfc1p = pool("wfc1p", 7)
        wfc2p = pool("wfc2p", 20)

        psB = pool("psB", 3, space="PSUM")
        psC = pool("psC", 3, space="PSUM")

        # constants
        ones16 = const.tile([128, 1], F16)
        nc.vector.memset(ones16[:], 1.0)
        onesr = const.tile([1, 64], F16)
        nc.vector.memset(onesr[:], 1.0)
        eps_t = const.tile([1, 1], F32)
        nc.vector.memset(eps_t[:], EPS_LN)
        idn = None  # placeholder

        # static inputs
        patches_sb = const.tile([PIX, T], F16)
        nc.sync.dma_start(out=patches_sb[:], in_=patchesT[:])
        wpatch_sb = const.tile([PIX, D], F16)
        nc.sync.dma_start(out=wpatch_sb[:], in_=wpatchT[:])
        pos_sb = const.tile([128, NCH, T], F32)
        nc.sync.dma_start(out=pos_sb[:], in_=posT.rearrange("c p t -> p c t"))
        m_sb = const.tile([KT, BL], F16)
        nc.sync.dma_start(out=m_sb[:], in_=mvec.rearrange("b t -> t b"))
        m32_sb = const.tile([KT, BL], F32)
        nc.vector.tensor_copy(m32_sb[:], m_sb[:])
        mf_sb = const.tile([1, BL, 2, 6 * KT], F32)
        nc.sync.dma_start(out=mf_sb[:], in_=mvecf[:])

        # residual stream, feature-major fp32
        H = hp.tile([128, NCH, T], F32)

        # ---- patch embed + pos add ----
        for grp in range(2):
            ps3 = psB.tile([128, 3, T], F32, tag="psB", name="pe3")
            for i in range(3):
                c = 3 * grp + i
                nc.tensor.matmul(ps3[:, i, :], wpatch_sb[:, 128 * c:128 * (c + 1)],
                                 patches_sb[:], start=(i == 0), stop=(i == 2))
            nc.vector.tensor_add(H[:, 3 * grp:3 * (grp + 1), :], ps3[:, :, :],
                                 pos_sb[:, 3 * grp:3 * (grp + 1), :])

        def layernorm(src, out_dt, y_pool):
            """src: [128, NCH, T] fp32 -> normalized tile in out_dt.

            Stats via ones-matmul over an fp16 [x | x^2] staging tile;
            rsqrt(var+eps) via quake-seed + 3 Newton steps entirely on DVE
            (keeps ACT's table stream to exactly exp-set / gelu-set);
            scale+shift broadcast with one gpsimd op; apply per chunk so
            the first downstream matmul unblocks after ~2 small DVE ops.
            """
            lnin = lnp.tile([128, 2, NCH, T], F16, tag="lnin")
            st = psC.tile([1, 2, T], F32, tag="psC", name="st")
            for bk in range(2):
                sl = slice(3 * bk, 3 * bk + 3)
                nc.vector.tensor_copy(lnin[:, 0, sl, :], src[:, sl, :])
            for bk in range(2):
                sl = slice(3 * bk, 3 * bk + 3)
                # x^2 on ACT (Square lives in every table set) so the DVE
                # only stands between the residual add and the first matmul
                nc.scalar.activation(lnin[:, 1, sl, :], src[:, sl, :], AF.Square)
            for c in range(NCH):
                nc.tensor.matmul(st[:], ones16[:, 0:1], lnin[:, :, c, :],
                                 start=(c == 0), stop=(c == NCH - 1))
            mean = tinyp.tile([1, T], F32, tag="tiny")
            nc.vector.tensor_scalar_mul(mean[:], st[0:1, 0, :], 1.0 / D)
            msq = tinyp.tile([1, T], F32, tag="tiny")
            nc.vector.tensor_mul(msq[:], mean[:], mean[:])
            v = tinyp.tile([1, T], F32, tag="tiny")
            nc.vector.scalar_tensor_tensor(v[:], st[0:1, 1, :], 1.0 / D, msq[:],
                                           op0=OP.mult, op1=OP.subtract)
            # rstd = 1/sqrt(v+eps) via quake seed + 2 Newton steps, all on
            # DVE: keeps ACT's table stream strictly exp-set/gelu-set and
            # (thanks to the K=1 mean-correction rows) this chain only gates
            # the psum drains, not the matmul stream.
            nc.vector.tensor_scalar_add(v[:], v[:], EPS_LN)
            seedi = tinyp.tile([1, T], mybir.dt.int32, tag="tiny")
            nc.vector.tensor_scalar(seedi[:], v[:].bitcast(mybir.dt.int32),
                                    1, None, op0=OP.arith_shift_right)
            nc.vector.tensor_scalar(seedi[:], seedi[:], 0x5F3759DF, -1,
                                    op0=OP.subtract, op1=OP.mult)
            anb = medp.tile([1, 2, T], F32, tag="anb", bufs=3)
            yy = tinyp.tile([1, T], F32, tag="tiny")
            t = tinyp.tile([1, T], F32, tag="tiny")
            cur = seedi[:].bitcast(F32)
            for it in range(2):
                nc.vector.tensor_mul(t[:], cur, cur)
                nc.vector.scalar_tensor_tensor(t[:], t[:], -0.5, v[:],
                                               op0=OP.mult, op1=OP.mult)
                dst = anb[0:1, 0, :] if it == 1 else yy[:]
                nc.vector.scalar_tensor_tensor(dst, t[:], 1.5, cur,
                                               op0=OP.add, op1=OP.mult)
                cur = yy[:]
            nc.vector.tensor_copy(anb[0:1, 1, :], mean[:])
            mu16 = tinyp.tile([1, T], F16, tag="tiny16")
            nc.vector.tensor_copy(mu16[:], mean[:])
            anb_b = bcp.tile([128, 2, T], F32, tag="bc")
            nc.gpsimd.partition_broadcast(anb_b[:], anb[:])
            rstd_b, mu_b = anb_b[:, 0, :], anb_b[:, 1, :]
            y = None
            if y_pool is not None:
                tmp = tmpp.tile([128, NCH, T], F32, tag="tmp")
                y = y_pool.tile([128, NCH, T], out_dt, tag=f"y{out_dt}",
                                bufs=2 if out_dt == F16 else 1)
                nc.vector.scalar_tensor_tensor(tmp[:], src[:, :, :], 1.0,
                                               bfree(mu_b, NCH),
                                               op0=OP.mult, op1=OP.subtract)
                nc.vector.tensor_mul(y[:], tmp[:], bfree(rstd_b, NCH))
            return y, lnin, mu16, rstd_b

        for l in range(depth):
            # weight loads for this layer (emitted first so DMA starts early)
            wsq_t = wsump.tile([1, 2 * D], F16, tag="wsq")
            nc.sync.dma_start(out=wsq_t[:], in_=wsqn[l:l + 1, :])
            wf1_t = wsump.tile([1, MLP], F16, tag="wf1")
            nc.sync.dma_start(out=wf1_t[:], in_=wsf1n[l:l + 1, :])
            wqkv = [wqkvp.tile([128, 3 * D], F16, tag="wqkv", name="wqkv") for _ in range(NCH)]
            for k in range(NCH):
                nc.sync.dma_start(out=wqkv[k][:], in_=wqkvT[l, 128 * k:128 * (k + 1), :])
            wproj = [wprojp.tile([128, D], F16, tag="wproj", name="wproj") for _ in range(NCH)]
            for k in range(NCH):
                nc.sync.dma_start(out=wproj[k][:], in_=wprojT[l, 128 * k:128 * (k + 1), :])
            wfc1 = [wfc1p.tile([128, MLP], F16, tag="wfc1", name="wfc1") for _ in range(NCH)]
            for k in range(NCH):
                nc.sync.dma_start(out=wfc1[k][:], in_=wfc1T[l, 128 * k:128 * (k + 1), :])
            wfc2 = [wfc2p.tile([128, D], F16, tag="wfc2", name="wfc2") for _ in range(MCH)]
            for k in range(MCH):
                nc.sync.dma_start(out=wfc2[k][:], in_=wfc2T[l, 128 * k:128 * (k + 1), :])


            # ---- LN1 ----
            y1, lnin1, mu16_1, rstd1_b = layernorm(H, F16, yp)

            # ---- QKV: Q,K feature-major ----
            qk16 = qkp.tile([128, 2 * NCH, T], F16, tag="qk")
            for grp in range(4):
                ps3 = psB.tile([128, 3, T], F32, tag="psB", name="qk3")
                for i in range(3):
                    oc = 3 * grp + i
                    for k in range(NCH):
                        nc.tensor.matmul(ps3[:, i, :],
                                         wqkv[k][:, 128 * oc:128 * (oc + 1)],
                                         lnin1[:, 0, k, :],
                                         start=(k == 0), stop=False)
                    # mean correction: out += (-colsum W)[o] * mu[t] (K=1)
                    nc.tensor.matmul(ps3[:, i, :],
                                     wsq_t[0:1, 128 * oc:128 * (oc + 1)],
                                     mu16_1[:], start=False, stop=True)
                nc.vector.tensor_mul(qk16[:, 3 * grp:3 * (grp + 1), :],
                                     ps3[:, :, :], bfree(rstd1_b, 3))

            # ---- V token-major per image ----
            v16 = []
            for b in range(BL):
                vps0 = psC.tile([KT, 512], F32, tag="psC")
                vps1 = psC.tile([KT, 512], F32, tag="psC")
                for k in range(NCH):
                    nc.tensor.matmul(vps0[:, 0:512],
                                     y1[:, k, KT * b:KT * (b + 1)],
                                     wqkv[k][:, 2 * D:2 * D + 512],
                                     start=(k == 0), stop=(k == NCH - 1))
                for k in range(NCH):
                    nc.tensor.matmul(vps1[:, 0:256],
                                     y1[:, k, KT * b:KT * (b + 1)],
                                     wqkv[k][:, 2 * D + 512:3 * D],
                                     start=(k == 0), stop=(k == NCH - 1))
                v = vp.tile([KT, D], F16, tag="v")
                nc.vector.tensor_scalar_mul(v[:, 0:512], vps0[:, 0:512],
                                            m32_sb[:, b:b + 1])
                nc.vector.tensor_scalar_mul(v[:, 512:768], vps1[:, 0:256],
                                            m32_sb[:, b:b + 1])
                v16.append(v)

            # ---- attention; images interleaved, heads grouped by parity.
            # PV runs on the raw exp(S) values: the attn mask is folded into
            # the V drain (row scale) and the 1/rowsum normalization into the
            # ot16 assembly multiply, so the softmax scalar chain never
            # blocks the PE stream.
            ot16 = otp.tile([128, NCH, T], F16, tag="ot")
            e16s = []
            for b in range(BL):
                e16 = ep.tile([KT, 2, 6 * KT], F16, tag="e")
                for g in range(2):
                    sps = psC.tile([KT, 512], F32, tag="psC")
                    for j in range(6):
                        nc.tensor.matmul(
                            sps[:, KT * j:KT * (j + 1)],
                            qk16[64 * g:64 * (g + 1), 6 + j, KT * b:KT * (b + 1)],
                            qk16[64 * g:64 * (g + 1), j, KT * b:KT * (b + 1)],
                            start=True, stop=True)
                    nc.scalar.activation(e16[:, g, :], sps[:, 0:6 * KT],
                                         AF.Exp, scale=SCALE)
                e16s.append(e16)
            for b in range(BL):
                e16 = e16s[b]
                rr = medp.tile([1, 2, 6 * KT], F32, tag="med")
                for g in range(2):
                    rps = psC.tile([1, 512], F32, tag="psC")
                    nc.tensor.matmul(rps[0:1, 0:6 * KT], m_sb[:, b:b + 1],
                                     e16[:, g, :], start=True, stop=True)
                    nc.vector.tensor_scalar_add(rps[0:1, 0:6 * KT],
                                                rps[0:1, 0:6 * KT], 1e-9)
                    nc.vector.reciprocal(rr[0:1, g, :], rps[0:1, 0:6 * KT])
                # rr *= m(t1): masked queries output 0 like the reference
                nc.vector.tensor_mul(rr[:], rr[:], mf_sb[0:1, b, :, :])
                rrb = bcp.tile([64, 2, 6 * KT], F32, tag="rb")
                nc.gpsimd.partition_broadcast(rrb[:], rr[:])
                for g in range(2):
                    ops = psC.tile([64, 512], F32, tag="psC")
                    for j in range(6):
                        nc.tensor.matmul(
                            ops[:, KT * j:KT * (j + 1)],
                            v16[b][:, 128 * j + 64 * g:128 * j + 64 * g + 64],
                            e16[:, g, KT * j:KT * (j + 1)],
                            start=True, stop=True)
                    nc.vector.tensor_mul(
                        ot16[64 * g:64 * (g + 1), :, KT * b:KT * (b + 1)],
                        ops[:, 0:6 * KT].rearrange("p (j t) -> p j t", j=6),
                        rrb[:, g, :].rearrange("p (j t) -> p j t", j=6))

            # ---- proj + residual ----
            # proj split by image: img0's half streams on PE while img1's
            # softmax scalar chain is still finishing
            pj = [psB.tile([128, 3, T], F32, tag="psB", name="pj3")
                  for _ in range(2)]
            for b in range(BL):
                cs = slice(KT * b, KT * (b + 1))
                for grp in range(2):
                    for i in range(3):
                        oc = 3 * grp + i
                        for k in range(NCH):
                            nc.tensor.matmul(pj[grp][:, i, cs],
                                             wproj[k][:, 128 * oc:128 * (oc + 1)],
                                             ot16[:, k, cs],
                                             start=(k == 0 and b == 0 and i == 0),
                                             stop=(k == NCH - 1 and b == BL - 1
                                                   and i == 2))
            for grp in range(2):
                sl = slice(3 * grp, 3 * (grp + 1))
                nc.vector.tensor_add(H[:, sl, :], H[:, sl, :], pj[grp][:, :, :])

            # ---- LN2 + MLP ----
            _, lnin2, mu16_2, rstd2_b = layernorm(H, F16, None)
            g16 = gp.tile([128, MCH, T], F16, tag="g")
            for grp in range(MCH // 3):
                ps3 = psB.tile([128, 3, T], F32, tag="psB")
                for i in range(3):
                    oc = 3 * grp + i
                    for k in range(NCH):
                        nc.tensor.matmul(ps3[:, i, :],
                                         wfc1[k][:, 128 * oc:128 * (oc + 1)],
                                         lnin2[:, 0, k, :],
                                         start=(k == 0), stop=False)
                    nc.tensor.matmul(ps3[:, i, :],
                                     wf1_t[0:1, 128 * oc:128 * (oc + 1)],
                                     mu16_2[:], start=False, stop=True)
                nc.vector.tensor_mul(ps3[:, :, :], ps3[:, :, :],
                                     bfree(rstd2_b, 3))
                nc.scalar.activation(g16[:, 3 * grp:3 * (grp + 1), :], ps3[:, :, :],
                                     AF.Gelu)
            # fc2 with k OUTER so each weight k-tile dies right after its 6
            # matmuls -> the next layer's fc2 DMA streams during this stage.
            # All 6 output accumulators live in one 2-bank psum tile
            # (3 x 152 fp32 = 1824B per bank, matmuls stay within a bank).
            # fc2 in two k-halves (o-outer inside): weight k-tiles die at
            # half boundaries so next-layer fc2 DMA streams during this
            # stage, while H chunks still finalize progressively in half 2
            # (so the next LN1 overlaps the fc2 tail).
            acc2 = psB.tile([128, 2, 512], F32, tag="psB2", bufs=1)
            a2 = lambda oc: acc2[:, oc // 3, T * (oc % 3):T * (oc % 3) + T]
            KH = MCH // 2
            for half in range(2):
                for oc in range(NCH):
                    for kk in range(KH):
                        k = half * KH + kk
                        nc.tensor.matmul(a2(oc),
                                         wfc2[k][:, 128 * oc:128 * (oc + 1)],
                                         g16[:, k, :],
                                         start=(k == 0 and oc % 3 == 0),
                                         stop=(k == MCH - 1 and oc % 3 == 2))
                    if half == 1 and oc % 3 == 2:
                        bank = oc // 3
                        sl = slice(3 * bank, 3 * bank + 3)
                        src_ap = acc2[:, bank, 0:3 * T].rearrange(
                            "p (i t) -> p i t", i=3)
                        nc.vector.tensor_add(H[:, sl, :], H[:, sl, :], src_ap)

        # ---- final LN (fp32 out) + store ----
        yf, _, _, _ = layernorm(H, F32, yp)
        for c in range(NCH):
            nc.sync.dma_start(out=out_d[c], in_=yf[:, c, :])

    nc.compile()
    return nc


def prep_inputs(inputs, depth=DEPTH):
    """Host-side marshalling. Returns per-core in_maps list."""
    g = {k: np.asarray(v) for k, v in inputs.items()}
    x = g["x"].astype(np.float32)
    noise = g["noise"].astype(np.float32)
    attn_mask = g["attn_mask"].astype(np.float32)
    ids_y = g["pos_embed_y_ids"].astype(np.int64)

    ids_shuffle = np.argsort(noise, axis=1, kind="stable")
    ids_keep = ids_shuffle[:, :LEN_KEEP]                      # (B, 75)

    patches = x.reshape(B, GH, GW, Q_).reshape(B, L, Q_)      # (B, 300, 100)
    mask_l = attn_mask.reshape(B, L)

    # pos vector per patch: [pos_y(384) | pos_x(384) * mask]
    pos_y = g["pos_y_table"].astype(np.float32)               # (13, 384)
    pos_x = g["pos_embed_x"].astype(np.float32)[0]            # (26, 384)
    ids_y_l = ids_y.reshape(B, L)
    gw_idx = np.tile(np.arange(GW), GH)                       # (300,)
    pos_full = np.zeros((B, L, D), np.float32)
    pos_full[:, :, :D // 2] = pos_y[ids_y_l]
    pos_full[:, :, D // 2:] = mask_l[:, :, None] * pos_x[gw_idx + 1][None]

    cls_vec = g["cls_token"].astype(np.float32).reshape(D).copy()
    cls_vec[D // 2:] += pos_x[0]

    wqkvT = np.ascontiguousarray(
        g["qkv_w"].astype(np.float32).transpose(0, 2, 1)[:depth]).astype(np.float16)
    wprojT = np.ascontiguousarray(
        g["proj_w"].astype(np.float32).transpose(0, 2, 1)[:depth]).astype(np.float16)
    wfc1T = np.ascontiguousarray(
        g["fc1_w"].astype(np.float32).transpose(0, 2, 1)[:depth]).astype(np.float16)
    wfc2T = np.ascontiguousarray(
        g["fc2_w"].astype(np.float32).transpose(0, 2, 1)[:depth]).astype(np.float16)
    wpatchT = np.ascontiguousarray(
        g["conv_w"].astype(np.float32).reshape(D, Q_).T).astype(np.float16)

    wsqn = -wqkvT[:, :, :2 * D].astype(np.float32).sum(axis=1).astype(np.float16)
    wsf1n = -wfc1T.astype(np.float32).sum(axis=1).astype(np.float16)

    in_maps = []
    for core in range(NCORES):
        patchesT = np.zeros((PIX, T), np.float16)
        posT = np.zeros((D, T), np.float32)
        mv = np.zeros((BL, KT), np.float16)
        for b in range(BL):
            img = core * BL + b
            sel = ids_keep[img]                               # (75,)
            patchesT[:, KT * b + 1:KT * (b + 1)] = patches[img, sel].T
            posT[:, KT * b] = cls_vec
            posT[:, KT * b + 1:KT * (b + 1)] = pos_full[img, sel].T
            mv[b, 0] = 1.0
            mv[b, 1:] = mask_l[img, np.sort(sel)]
        mvf = np.tile(mv.astype(np.float32)[:, None, :], (1, 12, 1)).reshape(
            1, BL, 2, 6 * KT)
        in_maps.append({
            "patchesT": patchesT,
            "posT": posT.reshape(NCH, 128, T),
            "mvec": mv,
            "mvecf": mvf,
            "wpatchT": wpatchT,
            "wqkvT": wqkvT,
            "wprojT": wprojT,
            "wfc1T": wfc1T,
            "wfc2T": wfc2T,
            "wsqn": wsqn,
            "wsf1n": wsf1n,
        })
    return in_maps


_NC_CACHE = {}


def kernel(**inputs):
    if "nc" not in _NC_CACHE:
        _NC_CACHE["nc"] = build()
    nc = _NC_CACHE["nc"]
    in_maps = prep_inputs(inputs)
    res = run_bass_kernel_spmd(nc, in_maps, list(range(NCORES)))
    # device output is feature-major (NCH, 128, T); untranspose on host
    outs = []
    for i in range(NCORES):
        a = res.results[i]["out"].reshape(D, T)          # (768, 152)
        outs.append(np.ascontiguousarray(a.T).reshape(BL, KT, D))
    return np.concatenate(outs, axis=0).astype(np.float32)



# revision 14
# speedup vs baseline: 1.0811x; 1.0811x over previous
"""MAE ViT encoder (nn_MaskedAutoencoderViT) Trainium2 Bass kernel.

Strategy: data-parallel over batch (16 images -> 8 cores x 2 images).
Feature-major activation layout on chip: activations stored transposed as
[128 partitions (d chunk), 6 chunks, 152 tokens] so every matmul is
weight-stationary (lhsT = 128x128 weight tile, rhs = activation columns)
with zero on-device transposes.  Attention is computed in transposed form
(S^T = (K^T)-stationary @ Q^T), softmax uses the structure
exp(att)/ (sum + 1e-9) (the reference's global-max subtraction cancels in
the normalization up to ~1e-10 relative, far below fp32 noise).
Matmul operands in fp16 (full PE rate, 11-bit mantissa), accumulation and
residual stream in fp32.

Host side does only data marshalling: noise argsort, patch gather,
pos-embed gathers, weight transposition + fp16 cast.
"""
import numpy as np
from contextlib import ExitStack

import concourse.bass as bass
import concourse.bacc as bacc
import concourse.mybir as mybir
import concourse.tile as tile
import bass_rust as _bass_rust
from concourse.bass_utils import run_bass_kernel_spmd
from concourse.hw_specs import get_activation_tables


class _Bacc(bacc.Bacc):
    """Bacc whose ACT-table-load pass prefers multi-function sets.

    The stock pass picks the first table set containing each activation
    function, which sends Ln to `natural_log` and Exp to `exp_and_others`
    and thrashes the table RAM inside every layernorm.  Reordering the
    set dict so `natural_log_exp_and_others` comes first makes Ln and Exp
    share one resident set (2 loads per layer total: exp-set <-> gelu-set).
    """

    def insert_act_table_loads(self):
        has_activation = any(
            isinstance(i, mybir.InstActivation)
            for b in self.main_func.blocks
            for i in b.instructions
        )
        if not has_activation:
            return
        tabs = dict(get_activation_tables(self.m.arch))
        pref = ["natural_log_exp_and_others", "gelu_and_others"]
        ordered = {k: tabs[k] for k in pref if k in tabs}
        ordered.update({k: v for k, v in tabs.items() if k not in ordered})
        _bass_rust.insert_act_table_loads(self, list(ordered.items()))

F16 = mybir.dt.float16
F32 = mybir.dt.float32
AF = mybir.ActivationFunctionType
OP = mybir.AluOpType

# --- model config (hardcoded from the problem spec) ---
B, C_IN, H_IN, W_IN = 16, 1, 12, 2500
P_, Q_ = 1, 100
D, NH, DEPTH = 768, 12, 12
GH, GW = 12, 25
L = GH * GW                      # 300
LEN_KEEP = 75
HD = D // NH                     # 64
SCALE = HD ** -0.5               # 0.125
EPS_LN = 1e-5
MLP = 4 * D                      # 3072

NCORES = 8
BL = B // NCORES                 # 2 images per core
KT = 1 + LEN_KEEP                # 76 tokens per image
T = BL * KT                      # 152 token columns per core
NCH = D // 128                   # 6 feature chunks
MCH = MLP // 128                 # 24 mlp chunks
PIX = P_ * Q_                    # 100 pixels per patch


def bfree(ap, n, at=1):
    """Insert a 0-step (broadcast) free dim of size n at position `at`."""
    new_ap = list(ap.ap[:at]) + [[0, n]] + list(ap.ap[at:])
    return bass.AP(tensor=ap.tensor, offset=ap.offset, ap=new_ap)


def build(depth=DEPTH):
    nc = bacc.Bacc("TRN2", target_bir_lowering=False, debug=False,
                   num_devices=NCORES)

    # DRAM I/O
    patchesT = nc.dram_tensor("patchesT", [PIX, T], F16, kind="ExternalInput").ap()
    posT = nc.dram_tensor("posT", [NCH, 128, T], F16, kind="ExternalInput").ap()
    mvec = nc.dram_tensor("mvec", [BL, KT], F16, kind="ExternalInput").ap()
    mrow = nc.dram_tensor("mrow", [1, BL, KT], F16, kind="ExternalInput").ap()
    wpatchT = nc.dram_tensor("wpatchT", [PIX, D], F16, kind="ExternalInput").ap()
    wqkvT = nc.dram_tensor("wqkvT", [depth, D, 3 * D], F16, kind="ExternalInput").ap()
    wprojT = nc.dram_tensor("wprojT", [depth, D, D], F16, kind="ExternalInput").ap()
    wfc1T = nc.dram_tensor("wfc1T", [depth, D, MLP], F16, kind="ExternalInput").ap()
    wfc2T = nc.dram_tensor("wfc2T", [depth, MLP, D], F16, kind="ExternalInput").ap()
    # per-layer small vectors: [colsum(-wqkv[:2D]) | colsum(-wfc1)]
    wsmall = nc.dram_tensor("wsmall", [depth, 2 * D + MLP], F16, kind="ExternalInput").ap()
    out_d = nc.dram_tensor("out", [NCH, 128, T], F32, kind="ExternalOutput").ap()

    with tile.TileContext(nc) as tc, ExitStack() as ctx:
        pool = lambda name, bufs, **kw: ctx.enter_context(
            tc.tile_pool(name=name, bufs=bufs, **kw))

        const = pool("const", 1)
        hp = pool("hp", 1)
        lnp = pool("lnp", 1)
        yp = pool("yp", 1)
        qkp = pool("qkp", 1)
        vp = pool("vp", 2)
        ep = pool("ep", 2)
        otp = pool("otp", 1)
        gp = pool("gp", 1)
        bcp = pool("bcp", 2)
        tinyp = pool("tinyp", 5)
        medp = pool("medp", 2)
        wsump = pool("wsump", 2)
        wqkvp = pool("wqkvp", 2)
        wprojp = pool("wprojp", 1)
        wfc1p = pool("wfc1p", 1)
        wfc2p = pool("wfc2p", 1)

        psB = pool("psB", 3, space="PSUM")
        psC = pool("psC", 3, space="PSUM")

        # constants
        ones16 = const.tile([128, 1], F16)
        nc.vector.memset(ones16[:], 1.0)
        onesr = const.tile([1, 64], F16)
        nc.vector.memset(onesr[:], 1.0)
        eps_t = const.tile([1, 1], F32)
        nc.vector.memset(eps_t[:], EPS_LN)
        idn = None  # placeholder

        # static inputs
        patches_sb = const.tile([PIX, T], F16)
        nc.sync.dma_start(out=patches_sb[:], in_=patchesT[:])
        wpatch_sb = const.tile([PIX, D], F16)
        nc.sync.dma_start(out=wpatch_sb[:], in_=wpatchT[:])
        pos_sb = const.tile([128, NCH, T], F16)
        nc.sync.dma_start(out=pos_sb[:], in_=posT.rearrange("c p t -> p c t"))
        m_sb = const.tile([KT, BL], F16)
        nc.sync.dma_start(out=m_sb[:], in_=mvec.rearrange("b t -> t b"))
        m32_sb = const.tile([KT, BL], F32)
        nc.vector.tensor_copy(m32_sb[:], m_sb[:])
        mrow_sb = const.tile([1, BL, KT], F16)
        nc.sync.dma_start(out=mrow_sb[:], in_=mrow[:])

        # residual stream, feature-major fp32
        H = hp.tile([128, NCH, T], F32)

        # ---- patch embed + pos add ----
        for grp in range(2):
            ps3 = psB.tile([128, 3, T], F32, tag="psB", name="pe3")
            for i in range(3):
                c = 3 * grp + i
                nc.tensor.matmul(ps3[:, i, :], wpatch_sb[:, 128 * c:128 * (c + 1)],
                                 patches_sb[:], start=(i == 0), stop=(i == 2))
            nc.vector.tensor_add(H[:, 3 * grp:3 * (grp + 1), :], ps3[:, :, :],
                                 pos_sb[:, 3 * grp:3 * (grp + 1), :])

        def layernorm(src, out_dt, y_pool):
            """src: [128, NCH, T] fp32 -> normalized tile in out_dt.

            Stats via ones-matmul over an fp16 [x | x^2] staging tile;
            rsqrt(var+eps) via quake-seed + 3 Newton steps entirely on DVE
            (keeps ACT's table stream to exactly exp-set / gelu-set);
            scale+shift broadcast with one gpsimd op; apply per chunk so
            the first downstream matmul unblocks after ~2 small DVE ops.
            """
            lnin = lnp.tile([128, 2, NCH, T], F16, tag="lnin")
            st = psC.tile([1, 2, T], F32, tag="psC", name="st")
            for bk in range(2):
                sl = slice(3 * bk, 3 * bk + 3)
                nc.vector.tensor_copy(lnin[:, 0, sl, :], src[:, sl, :])
            for bk in range(2):
                sl = slice(3 * bk, 3 * bk + 3)
                # x^2 on ACT (Square lives in every table set) so the DVE
                # only stands between the residual add and the first matmul
                nc.scalar.activation(lnin[:, 1, sl, :], src[:, sl, :], AF.Square)
            for c in range(NCH):
                nc.tensor.matmul(st[:], ones16[:, 0:1], lnin[:, :, c, :],
                                 start=(c == 0), stop=(c == NCH - 1))
            mean = tinyp.tile([1, T], F32, tag="tiny")
            nc.vector.tensor_scalar_mul(mean[:], st[0:1, 0, :], 1.0 / D)
            msq = tinyp.tile([1, T], F32, tag="tiny")
            nc.vector.tensor_mul(msq[:], mean[:], mean[:])
            v = tinyp.tile([1, T], F32, tag="tiny")
            nc.vector.scalar_tensor_tensor(v[:], st[0:1, 1, :], 1.0 / D, msq[:],
                                           op0=OP.mult, op1=OP.subtract)
            # rstd = 1/sqrt(v+eps) via quake seed + 2 Newton steps, all on
            # DVE: keeps ACT's table stream strictly exp-set/gelu-set and
            # (thanks to the K=1 mean-correction rows) this chain only gates
            # the psum drains, not the matmul stream.
            nc.vector.tensor_scalar_add(v[:], v[:], EPS_LN)
            seedi = tinyp.tile([1, T], mybir.dt.int32, tag="tiny")
            nc.vector.tensor_scalar(seedi[:], v[:].bitcast(mybir.dt.int32),
                                    1, None, op0=OP.arith_shift_right)
            nc.vector.tensor_scalar(seedi[:], seedi[:], 0x5F3759DF, -1,
                                    op0=OP.subtract, op1=OP.mult)
            anb = medp.tile([1, 2, T], F32, tag="anb", bufs=3)
            yy = tinyp.tile([1, T], F32, tag="tiny")
            t = tinyp.tile([1, T], F32, tag="tiny")
            cur = seedi[:].bitcast(F32)
            for it in range(2):
                nc.vector.tensor_mul(t[:], cur, cur)
                nc.vector.scalar_tensor_tensor(t[:], t[:], -0.5, v[:],
                                               op0=OP.mult, op1=OP.mult)
                dst = anb[0:1, 0, :] if it == 1 else yy[:]
                nc.vector.scalar_tensor_tensor(dst, t[:], 1.5, cur,
                                               op0=OP.add, op1=OP.mult)
                cur = yy[:]
            nc.vector.tensor_copy(anb[0:1, 1, :], mean[:])
            mu16 = tinyp.tile([1, T], F16, tag="tiny16", bufs=2)
            nc.vector.tensor_copy(mu16[:], mean[:])
            anb_b = bcp.tile([128, 2, T], F32, tag="bc")
            nc.gpsimd.partition_broadcast(anb_b[:], anb[:])
            rstd_b, mu_b = anb_b[:, 0, :], anb_b[:, 1, :]
            y = None
            if y_pool is not None:
                y = y_pool.tile([128, NCH, T], out_dt, tag=f"y{out_dt}", bufs=1)
                nc.vector.scalar_tensor_tensor(y[:], src[:, :, :], 1.0,
                                               bfree(mu_b, NCH),
                                               op0=OP.mult, op1=OP.subtract)
                nc.vector.tensor_mul(y[:], y[:], bfree(rstd_b, NCH))
            return y, lnin, mu16, rstd_b

        # wqkv + wsmall are prefetched one layer ahead (bufs=2); proj/fc1/fc2
        # stream within their own layer (bufs=1), issued in consumption order
        # so the single DMA pipe never idles and never head-of-line blocks.
        def issue_qkv(l):
            sm = wsump.tile([1, 2 * D + MLP], F16, tag="wsm", bufs=2, name="wsm")
            nc.sync.dma_start(out=sm[:], in_=wsmall[l:l + 1, :])
            qv = wqkvp.tile([128, NCH, 3 * D], F16, tag="wqkv", name="wqkv")
            nc.sync.dma_start(out=qv[:], in_=wqkvT[l].rearrange("(k p) o -> p k o", p=128))
            return sm, qv

        pending = issue_qkv(0)

        for l in range(depth):
            wsm_t, wqkv_t = pending
            wsq_t = wsm_t[:, 0:2 * D]
            wf1_t = wsm_t[:, 2 * D:]
            wqkv = [wqkv_t[:, k, :] for k in range(NCH)]

            wproj_t = wprojp.tile([128, NCH, D], F16, tag="wproj", name="wproj")
            nc.sync.dma_start(out=wproj_t[:], in_=wprojT[l].rearrange("(k p) o -> p k o", p=128))
            wproj = [wproj_t[:, k, :] for k in range(NCH)]
            wfc1_t = wfc1p.tile([128, NCH, MLP], F16, tag="wfc1", name="wfc1")
            nc.sync.dma_start(out=wfc1_t[:], in_=wfc1T[l].rearrange("(k p) o -> p k o", p=128))
            wfc1 = [wfc1_t[:, k, :] for k in range(NCH)]
            wfc2_t = wfc2p.tile([128, MCH, D], F16, tag="wfc2", name="wfc2")
            nc.sync.dma_start(out=wfc2_t[:], in_=wfc2T[l].rearrange("(k p) o -> p k o", p=128))
            wfc2 = [wfc2_t[:, k, :] for k in range(MCH)]
            if l + 1 < depth:
                pending = issue_qkv(l + 1)


            # ---- LN1 ----
            y1, lnin1, mu16_1, rstd1_b = layernorm(H, F16, yp)

            # ---- QKV: Q,K feature-major ----
            qk16 = qkp.tile([128, 2 * NCH, T], F16, tag="qk")
            for grp in range(4):
                ps3 = psB.tile([128, 3, T], F32, tag="psB", name="qk3")
                for i in range(3):
                    oc = 3 * grp + i
                    for k in range(NCH):
                        nc.tensor.matmul(ps3[:, i, :],
                                         wqkv[k][:, 128 * oc:128 * (oc + 1)],
                                         lnin1[:, 0, k, :],
                                         start=(k == 0), stop=False)
                    # mean correction: out += (-colsum W)[o] * mu[t] (K=1)
                    nc.tensor.matmul(ps3[:, i, :],
                                     wsq_t[0:1, 128 * oc:128 * (oc + 1)],
                                     mu16_1[:], start=False, stop=True)
                nc.vector.tensor_mul(qk16[:, 3 * grp:3 * (grp + 1), :],
                                     ps3[:, :, :], bfree(rstd1_b, 3))

            # ---- V token-major per image ----
            v16 = []
            for b in range(BL):
                vps0 = psC.tile([KT, 512], F32, tag="psC")
                vps1 = psC.tile([KT, 512], F32, tag="psC")
                for k in range(NCH):
                    nc.tensor.matmul(vps0[:, 0:512],
                                     y1[:, k, KT * b:KT * (b + 1)],
                                     wqkv[k][:, 2 * D:2 * D + 512],
                                     start=(k == 0), stop=(k == NCH - 1))
                for k in range(NCH):
                    nc.tensor.matmul(vps1[:, 0:256],
                                     y1[:, k, KT * b:KT * (b + 1)],
                                     wqkv[k][:, 2 * D + 512:3 * D],
                                     start=(k == 0), stop=(k == NCH - 1))
                v = vp.tile([KT, D], F16, tag="v")
                nc.vector.tensor_scalar_mul(v[:, 0:512], vps0[:, 0:512],
                                            m32_sb[:, b:b + 1])
                nc.vector.tensor_scalar_mul(v[:, 512:768], vps1[:, 0:256],
                                            m32_sb[:, b:b + 1])
                v16.append(v)

            # ---- attention; images interleaved, heads grouped by parity.
            # PV runs on the raw exp(S) values: the attn mask is folded into
            # the V drain (row scale) and the 1/rowsum normalization into the
            # ot16 assembly multiply, so the softmax scalar chain never
            # blocks the PE stream.
            ot16 = otp.tile([128, NCH, T], F16, tag="ot")
            e16s = []
            for b in range(BL):
                e16 = ep.tile([KT, 2, 6 * KT], F16, tag="e")
                for g in range(2):
                    sps = psC.tile([KT, 512], F32, tag="psC")
                    for j in range(6):
                        nc.tensor.matmul(
                            sps[:, KT * j:KT * (j + 1)],
                            qk16[64 * g:64 * (g + 1), 6 + j, KT * b:KT * (b + 1)],
                            qk16[64 * g:64 * (g + 1), j, KT * b:KT * (b + 1)],
                            start=True, stop=True)
                    nc.scalar.activation(e16[:, g, :], sps[:, 0:6 * KT],
                                         AF.Exp, scale=SCALE)
                e16s.append(e16)
            for b in range(BL):
                e16 = e16s[b]
                rr = medp.tile([1, 2, 6 * KT], F16, tag="med")
                for g in range(2):
                    rps = psC.tile([1, 512], F32, tag="psC")
                    nc.tensor.matmul(rps[0:1, 0:6 * KT], m_sb[:, b:b + 1],
                                     e16[:, g, :], start=True, stop=True)
                    nc.vector.tensor_scalar_add(rps[0:1, 0:6 * KT],
                                                rps[0:1, 0:6 * KT], 1e-9)
                    with nc.allow_low_precision(reason="1/rowsum feeds fp16 ot"):
                        nc.vector.reciprocal(rr[0:1, g, :], rps[0:1, 0:6 * KT])
                # rr *= m(t1): masked queries output 0 like the reference
                nc.vector.tensor_mul(
                    rr[:].rearrange("p x (j t) -> p x j t", j=6),
                    rr[:].rearrange("p x (j t) -> p x j t", j=6),
                    bfree(bfree(mrow_sb[0:1, b, :], 6), 2))
                rrb = bcp.tile([64, 2, 6 * KT], F16, tag="rb")
                nc.gpsimd.partition_broadcast(rrb[:], rr[:])
                for g in range(2):
                    ops = psC.tile([64, 512], F32, tag="psC")
                    for j in range(6):
                        nc.tensor.matmul(
                            ops[:, KT * j:KT * (j + 1)],
                            v16[b][:, 128 * j + 64 * g:128 * j + 64 * g + 64],
                            e16[:, g, KT * j:KT * (j + 1)],
                            start=True, stop=True)
                    nc.vector.tensor_mul(
                        ot16[64 * g:64 * (g + 1), :, KT * b:KT * (b + 1)],
                        ops[:, 0:6 * KT].rearrange("p (j t) -> p j t", j=6),
                        rrb[:, g, :].rearrange("p (j t) -> p j t", j=6))

            # ---- proj + residual ----
            # proj split by image: img0's half streams on PE while img1's
            # softmax scalar chain is still finishing
            pj = [psB.tile([128, 3, T], F32, tag="psB", name="pj3")
                  for _ in range(2)]
            for b in range(BL):
                cs = slice(KT * b, KT * (b + 1))
                for grp in range(2):
                    for i in range(3):
                        oc = 3 * grp + i
                        for k in range(NCH):
                            nc.tensor.matmul(pj[grp][:, i, cs],
                                             wproj[k][:, 128 * oc:128 * (oc + 1)],
                                             ot16[:, k, cs],
                                             start=(k == 0 and b == 0 and i == 0),
                                             stop=(k == NCH - 1 and b == BL - 1
                                                   and i == 2))
            for grp in range(2):
                sl = slice(3 * grp, 3 * (grp + 1))
                nc.vector.tensor_add(H[:, sl, :], H[:, sl, :], pj[grp][:, :, :])

            # ---- LN2 + MLP ----
            _, lnin2, mu16_2, rstd2_b = layernorm(H, F16, None)
            g16 = gp.tile([128, MCH, T], F16, tag="g")
            for grp in range(MCH // 3):
                ps3 = psB.tile([128, 3, T], F32, tag="psB")
                for i in range(3):
                    oc = 3 * grp + i
                    for k in range(NCH):
                        nc.tensor.matmul(ps3[:, i, :],
                                         wfc1[k][:, 128 * oc:128 * (oc + 1)],
                                         lnin2[:, 0, k, :],
                                         start=(k == 0), stop=False)
                    nc.tensor.matmul(ps3[:, i, :],
                                     wf1_t[0:1, 128 * oc:128 * (oc + 1)],
                                     mu16_2[:], start=False, stop=True)
                nc.vector.tensor_mul(ps3[:, :, :], ps3[:, :, :],
                                     bfree(rstd2_b, 3))
                nc.scalar.activation(g16[:, 3 * grp:3 * (grp + 1), :], ps3[:, :, :],
                                     AF.Gelu)
            # fc2 with k OUTER so each weight k-tile dies right after its 6
            # matmuls -> the next layer's fc2 DMA streams during this stage.
            # All 6 output accumulators live in one 2-bank psum tile
            # (3 x 152 fp32 = 1824B per bank, matmuls stay within a bank).
            # fc2 in two k-halves (o-outer inside): weight k-tiles die at
            # half boundaries so next-layer fc2 DMA streams during this
            # stage, while H chunks still finalize progressively in half 2
            # (so the next LN1 overlaps the fc2 tail).
            acc2 = psB.tile([128, 2, 512], F32, tag="psB2", bufs=1)
            a2 = lambda oc: acc2[:, oc // 3, T * (oc % 3):T * (oc % 3) + T]
            KH = MCH // 2
            for half in range(2):
                for oc in range(NCH):
                    for kk in range(KH):
                        k = half * KH + kk
                        nc.tensor.matmul(a2(oc),
                                         wfc2[k][:, 128 * oc:128 * (oc + 1)],
                                         g16[:, k, :],
                                         start=(k == 0 and oc % 3 == 0),
                                         stop=(k == MCH - 1 and oc % 3 == 2))
                    if half == 1 and oc % 3 == 2:
                        bank = oc // 3
                        sl = slice(3 * bank, 3 * bank + 3)
                        src_ap = acc2[:, bank, 0:3 * T].rearrange(
                            "p (i t) -> p i t", i=3)
                        nc.vector.tensor_add(H[:, sl, :], H[:, sl, :], src_ap)

        # ---- final LN (fp32 out) + store ----
        yf, _, _, _ = layernorm(H, F32, yp)
        for c in range(NCH):
            nc.sync.dma_start(out=out_d[c], in_=yf[:, c, :])

    nc.compile()
    return nc


def prep_inputs(inputs, depth=DEPTH):
    """Host-side marshalling. Returns per-core in_maps list."""
    g = {k: np.asarray(v) for k, v in inputs.items()}
    x = g["x"].astype(np.float32)
    noise = g["noise"].astype(np.float32)
    attn_mask = g["attn_mask"].astype(np.float32)
    ids_y = g["pos_embed_y_ids"].astype(np.int64)

    ids_shuffle = np.argsort(noise, axis=1, kind="stable")
    ids_keep = ids_shuffle[:, :LEN_KEEP]                      # (B, 75)

    patches = x.reshape(B, GH, GW, Q_).reshape(B, L, Q_)      # (B, 300, 100)
    mask_l = attn_mask.reshape(B, L)

    # pos vector per patch: [pos_y(384) | pos_x(384) * mask]
    pos_y = g["pos_y_table"].astype(np.float32)               # (13, 384)
    pos_x = g["pos_embed_x"].astype(np.float32)[0]            # (26, 384)
    ids_y_l = ids_y.reshape(B, L)
    gw_idx = np.tile(np.arange(GW), GH)                       # (300,)
    pos_full = np.zeros((B, L, D), np.float32)
    pos_full[:, :, :D // 2] = pos_y[ids_y_l]
    pos_full[:, :, D // 2:] = mask_l[:, :, None] * pos_x[gw_idx + 1][None]

    cls_vec = g["cls_token"].astype(np.float32).reshape(D).copy()
    cls_vec[D // 2:] += pos_x[0]

    wqkvT = np.ascontiguousarray(
        g["qkv_w"].astype(np.float32).transpose(0, 2, 1)[:depth]).astype(np.float16)
    wprojT = np.ascontiguousarray(
        g["proj_w"].astype(np.float32).transpose(0, 2, 1)[:depth]).astype(np.float16)
    wfc1T = np.ascontiguousarray(
        g["fc1_w"].astype(np.float32).transpose(0, 2, 1)[:depth]).astype(np.float16)
    wfc2T = np.ascontiguousarray(
        g["fc2_w"].astype(np.float32).transpose(0, 2, 1)[:depth]).astype(np.float16)
    wpatchT = np.ascontiguousarray(
        g["conv_w"].astype(np.float32).reshape(D, Q_).T).astype(np.float16)

    wsqn = -wqkvT[:, :, :2 * D].astype(np.float32).sum(axis=1).astype(np.float16)
    wsf1n = -wfc1T.astype(np.float32).sum(axis=1).astype(np.float16)
    wsmall = np.ascontiguousarray(np.concatenate([wsqn, wsf1n], axis=1))

    in_maps = []
    for core in range(NCORES):
        patchesT = np.zeros((PIX, T), np.float16)
        posT = np.zeros((D, T), np.float32)
        mv = np.zeros((BL, KT), np.float16)
        for b in range(BL):
            img = core * BL + b
            sel = ids_keep[img]                               # (75,)
            patchesT[:, KT * b + 1:KT * (b + 1)] = patches[img, sel].T
            posT[:, KT * b] = cls_vec
            posT[:, KT * b + 1:KT * (b + 1)] = pos_full[img, sel].T
            mv[b, 0] = 1.0
            mv[b, 1:] = mask_l[img, np.sort(sel)]
        in_maps.append({
            "patchesT": patchesT,
            "posT": posT.reshape(NCH, 128, T).astype(np.float16),
            "mvec": mv,
            "mrow": mv.reshape(1, BL, KT),
            "wpatchT": wpatchT,
            "wqkvT": wqkvT,
            "wprojT": wprojT,
            "wfc1T": wfc1T,
            "wfc2T": wfc2T,
            "wsmall": wsmall,
        })
    return in_maps


_NC_CACHE = {}


def kernel(**inputs):
    if "nc" not in _NC_CACHE:
        _NC_CACHE["nc"] = build()
    nc = _NC_CACHE["nc"]
    in_maps = prep_inputs(inputs)
    res = run_bass_kernel_spmd(nc, in_maps, list(range(NCORES)))
    # device output is feature-major (NCH, 128, T); untranspose on host
    outs = []
    for i in range(NCORES):
        a = res.results[i]["out"].reshape(D, T)          # (768, 152)
        outs.append(np.ascontiguousarray(a.T).reshape(BL, KT, D))
    return np.concatenate(outs, axis=0).astype(np.float32)

